# revision 9
# baseline (speedup 1.0000x reference)
"""Distributed Trainium2 kernel for the sparse-attention + depthwise-conv module.

Math: q/k are l2-normalized over the full spatial axis n and the score matrix
is a tiny [b,h,64,64], so the whole attention collapses through the per-batch
Gram matrix G = X^T X ([64,64]):
  S_raw[h] = Wk_h^T G Wq_h, kk = diag(Wk_h^T G Wk_h), qq = diag(Wq_h^T G Wq_h)
  attn = softmax(S_raw * rescale / sqrt(kk qq))
  Wtilde[h] = attn_h^T (Wp_h / rowsum),  Weff = Wv @ Wtilde   ([64,64] per b)
  out = depthwise_conv3x3(x) + X @ Weff + bp
Only G crosses cores (AllGather of 16KB bf16 + local sum: the 8-core AllGather
floor is ~2x lower than AllReduce's, and the payload is latency-bound).

Layout: x is uploaded batch-STACKED bf16 ([128part = b0 ch | b1 ch]) in four
10-row slices (2-row overlap) so transposes/conv matmuls start as slices land.
 - G: PE pair-transposes [128,128] blocks of a single image row (both batches
   at once) -> xt; G MMs accumulate b0/b1 concurrently in opposite PE column
   groups (g_ps [128,64]).
 - conv: block-diagonal [128,128] stationaries (diag(tap) per batch) compute
   both batches' 64 output channels in one K=128 matmul; 9 tap slots per
   2-row chunk + a folded Weff slot (late pairs) or a separate fix-up matmul
   + DVE add (early pairs).  Early pairs' conv work is AllGather-independent
   and fills the PE through the collective's ~20-35us latency, also keeping
   the HAM clock-gate warm (2.4GHz).
PE warmup matmuls on a zero tile run during the x DMA window so the clock
gate is already released when real work arrives.
"""

import os
import numpy as np
import ml_dtypes

BF = ml_dtypes.bfloat16
B, C, H, W = 2, 64, 256, 256
HEADS, D = 8, 64
INNER = HEADS * D          # 512
NCORES = 8
RPC = H // NCORES          # 32 output rows per core per batch
WP = 272                   # padded row length (16-elem multiple)
NLOC = RPC * W             # 8192 spatial positions per core per batch
NCHUNKS = NLOC // 512      # 16 row-pair chunks
SL_START = [0, 8, 16, 24]  # slice start rows (10 rows each, 2-row overlap)
SL_ROWS = 10
SL_FREE = SL_ROWS * WP     # 2720

_CACHE = {}


def _build():
    import concourse.bass as bass
    import concourse.bacc as bacc
    import concourse.mybir as mybir
    import concourse.tile as tile

    f32 = mybir.dt.float32
    bf16 = mybir.dt.bfloat16

    nc = bacc.Bacc("TRN2", target_bir_lowering=False, debug=False,
                   num_devices=NCORES)

    x_d = nc.dram_tensor("x", [128, 4 * SL_FREE], bf16,
                         kind="ExternalInput").ap()
    wq2_d = nc.dram_tensor("wq2", [128, INNER], bf16,
                           kind="ExternalInput").ap()
    wk2_d = nc.dram_tensor("wk2", [128, INNER], bf16,
                           kind="ExternalInput").ap()
    wvt_d = nc.dram_tensor("wvt", [128, 256], bf16, kind="ExternalInput").ap()
    wp2_d = nc.dram_tensor("wp2", [128, INNER], f32,
                           kind="ExternalInput").ap()
    tap9_d = nc.dram_tensor("tap9", [128, 9 * 128], bf16,
                            kind="ExternalInput").ap()
    ones2_d = nc.dram_tensor("ones2", [128, C], bf16,
                             kind="ExternalInput").ap()
    idn_d = nc.dram_tensor("idn", [128, 128], bf16, kind="ExternalInput").ap()
    bp_d = nc.dram_tensor("bp", [128, 1], f32, kind="ExternalInput").ap()
    rsc2_d = nc.dram_tensor("rsc2", [128, INNER], f32,
                            kind="ExternalInput").ap()
    out_d = nc.dram_tensor("out", [B * C, NLOC], bf16,
                           kind="ExternalOutput").ap()

    Act = mybir.ActivationFunctionType
    N_EARLY = int(os.environ.get("KERNEL_EARLY_PAIRS", "16"))
    N_WARM = int(os.environ.get("KERNEL_WARM_MMS", "55"))
    N_FILL = int(os.environ.get("KERNEL_FILL_MMS", "70"))

    with tile.TileContext(nc) as tc:
        with (
            tc.tile_pool(name="xp", bufs=1) as xpool,
            tc.tile_pool(name="wp", bufs=1) as wpool,
            tc.tile_pool(name="sp", bufs=1) as spool,
            tc.tile_pool(name="xt", bufs=4) as xtpool,
            tc.tile_pool(name="ob", bufs=1) as opool,
            tc.tile_pool(name="ps", bufs=1, space="PSUM") as pspool,
            tc.tile_pool(name="dr", bufs=1, space="DRAM") as drpool,
        ):
            # ---- PE warmup: ~40 small matmuls on a zeroed tile release the
            # HAM clock gate (~3.4us of activity) while the x DMA streams in.
            warm = spool.tile([64, 64], bf16, tag="warm")
            nc.vector.memset(warm[:], 0.0)
            warm_ps = pspool.tile([64, 64], f32, tag="tps", bufs=3,
                                  name="warm_ps")
            for i in range(N_WARM):
                nc.tensor.matmul(warm_ps[:], warm[:], warm[:],
                                 start=(i == 0), stop=(i == N_WARM - 1))

            # ---- x: 4 batch-stacked bf16 slices (10 rows each)
            slices = []
            for g in range(4):
                s = xpool.tile([128, SL_FREE], bf16, tag=f"s{g}",
                               name=f"s{g}")
                nc.gpsimd.dma_start(s[:], x_d[:, g * SL_FREE:(g + 1) * SL_FREE])
                slices.append(s)
            svs = [s[:, :].rearrange("p (r w) -> p r w", w=WP)
                   for s in slices]

            # ---- weights (idn gates the transposes -> first)
            idn_s = wpool.tile_from(idn_d)
            tap9_s = wpool.tile_from(tap9_d)
            bp_s = wpool.tile_from(bp_d)
            wq2_s = wpool.tile_from(wq2_d)
            wk2_s = wpool.tile_from(wk2_d)
            wvt_s = wpool.tile_from(wvt_d)
            wp2_s = wpool.tile_from(wp2_d)
            ones2_s = wpool.tile_from(ones2_d)
            rsc2_s = wpool.tile_from(rsc2_d)

            # ---- G = X^T X for both batches at once.
            # Transpose one image row x 128 cols of the stacked tile
            # ([128part=(b0|b1 ch), 128]) against I128; 4 transposes fill a
            # [128,512] psum -> bf16 xt; 8 G-matmuls per xt (4 blocks x 2
            # batches) accumulate b0 into PE cols 0:63 / b1 into 64:127
            # concurrently (opposite column groups, separate psum halves).
            def row_slice(r):
                g = max(0, (r - 2) // 8)
                return g, r - SL_START[g]

            g_ps = pspool.tile([128, 64], f32, tag="gps", name="g_ps")
            first = {0: True, 1: True}
            for grp in range(16):
                tp = pspool.tile([128, 512], f32, tag="tps", bufs=3,
                                 name=f"tp{grp}")
                for j in range(4):
                    t = grp * 4 + j          # 0..63
                    r = 1 + t // 2
                    xh = t % 2
                    g, lr = row_slice(r)
                    off = lr * WP + 1 + 128 * xh
                    nc.tensor.matmul(tp[:, j * 128:(j + 1) * 128],
                                     slices[g][:, off:off + 128], idn_s[:],
                                     start=True, stop=True,
                                     skip_group_check=True)
                xt = xtpool.tile([128, 512], bf16, tag="xt",
                                 name=f"xt{grp}")
                if grp % 2 == 0:
                    nc.vector.tensor_copy(xt[:], tp[:])
                else:
                    nc.scalar.copy(xt[:], tp[:])
                for j in range(4):
                    for b in range(B):
                        nc.tensor.matmul(
                            g_ps[b * 64:(b + 1) * 64, :],
                            xt[:, j * 128 + b * 64:j * 128 + (b + 1) * 64],
                            xt[:, j * 128 + b * 64:j * 128 + (b + 1) * 64],
                            start=first[b],
                            stop=(grp == 15 and j == 3),
                            skip_group_check=True,
                            tile_position=(0, b * 64),
                        )
                        first[b] = False

            # ---- AllGather G (bf16, 16KB) + local sum.  gcat[64,128] holds
            # b0|b1 side by side (ACT copies handle the partition crossing).
            gcat = spool.tile([64, 128], bf16, tag="gcat")
            nc.scalar.copy(gcat[:, 0:64], g_ps[0:64, :])
            nc.scalar.copy(gcat[:, 64:128], g_ps[64:128, :])
            g_in = drpool.tile([64, 128], bf16, tag="gin")
            g_out = drpool.tile([NCORES, 64, 128], bf16, tag="gout")
            nc.sync.dma_start(g_in[:], gcat[:])
            nc.gpsimd.collective_compute(
                "AllGather", mybir.AluOpType.bypass,
                replica_groups=[list(range(NCORES))],
                ins=[g_in.opt()], outs=[g_out.opt()],
            )
            g8 = spool.tile([64, NCORES * 128], bf16, tag="g8")
            nc.sync.dma_start(
                g8[:, :].rearrange("p (r c) -> p r c", c=128),
                g_out.rearrange("r p c -> p r c"))
            # local sum of the 8 gathered G blocks: 3-level add tree
            g4 = spool.tile([64, 512], bf16, tag="g4")
            nc.vector.tensor_add(g4[:], g8[:, 0:512], g8[:, 512:1024])
            g2 = spool.tile([64, 256], bf16, tag="g2")
            nc.vector.tensor_add(g2[:], g4[:, 0:256], g4[:, 256:512])
            gsum_bf = spool.tile([64, 128], bf16, tag="gsumbf")
            nc.vector.tensor_add(gsum_bf[:], g2[:, 0:128], g2[:, 128:256])

            # ---- conv (+ attention) chunk pairs: block-diagonal [128,128]
            # stationaries compute both batches' 64 out-channels in one
            # K=128 matmul.  9 tap slots; late pairs fold X@Weff as a 10th
            # slot, early pairs get a separate fix-up matmul + DVE add.
            ctr2 = []        # filled after head math
            osbs = {}
            early = set(range(N_EARLY))

            def gen1(ci):
                folded = ci not in early
                g = ci // 4
                ly0 = 2 * ci - SL_START[g]
                cps = pspool.tile([128, 512], f32, tag="conv", bufs=4,
                                  name=f"cps{ci}")
                nslot = 10 if folded else 9
                si = 0
                for dy in range(3):
                    for dx in range(3):
                        nc.tensor.matmul(
                            cps[:],
                            tap9_s[:, (dy * 3 + dx) * 128:
                                   (dy * 3 + dx + 1) * 128],
                            svs[g][:, ly0 + dy:ly0 + dy + 2, dx:dx + 256],
                            start=(si == 0), stop=(si == nslot - 1),
                            skip_group_check=True)
                        si += 1
                if folded:
                    nc.tensor.matmul(
                        cps[:], ctr2[0][:],
                        svs[g][:, ly0 + 1:ly0 + 3, 1:257],
                        start=False, stop=True, skip_group_check=True)
                osb = opool.tile([128, 512], bf16, tag="osb", bufs=16,
                                 name=f"osb{ci}")
                osbs[ci] = osb
                nc.scalar.activation(osb[:], cps[:], Act.Identity,
                                     bias=bp_s[:])
                if folded:
                    flush(ci)

            def gen2(ci):
                g = ci // 4
                ly0 = 2 * ci - SL_START[g]
                aps = pspool.tile([128, 512], f32, tag="tps", bufs=3,
                                  name=f"aps{ci}")
                nc.tensor.matmul(aps[:], ctr2[0][:],
                                 svs[g][:, ly0 + 1:ly0 + 3, 1:257],
                                 start=True, stop=True,
                                 skip_group_check=True)
                osb = osbs[ci]
                nc.vector.tensor_add(osb[:], osb[:], aps[:])
                flush(ci)

            def flush(ci):
                nc.gpsimd.dma_start(out_d[:, ci * 512:(ci + 1) * 512],
                                    osbs[ci][:])

            # AllGather-independent conv first: keeps the PE busy (and the
            # clock warm) through the collective's latency.
            for ci in sorted(early):
                gen1(ci)

            # ---- warm-fill: dependency-free matmuls sized to span the
            # AllGather's remaining latency so the HAM clock gate stays
            # released for the head chain + tail (env KERNEL_FILL_MMS).
            warm2 = spool.tile([64, 512], bf16, tag="warm2")
            nc.vector.memset(warm2[:], 0.0)
            fill_ps = pspool.tile([64, 512], f32, tag="tps", bufs=3,
                                  name="fill_ps")
            for i in range(N_FILL):
                nc.tensor.matmul(fill_ps[:], warm[:], warm2[:],
                                 start=(i == 0), stop=(i == N_FILL - 1))

            # ---- head math -> Weff, both batches merged: b0 lives on
            # partitions/PE-rows 0:63, b1 on 64:127 (doubled weights); each
            # elementwise/activation stage is ONE [128,512] op and the two
            # batches' matmuls run concurrently in opposite PE quadrants.
            def act_rsqrt(out, in_):
                # raw InstActivation: bass blocks ACT Rsqrt for accuracy, but
                # table accuracy (~1e-3) is far inside this kernel's 2e-2
                # budget and it replaces a 3.3us DVE Newton reciprocal.
                eng = nc.scalar
                return eng.add_instruction(mybir.InstActivation(
                    name=nc.get_next_instruction_name(),
                    func=Act.Rsqrt,
                    ins=[eng.lower_ap(in_),
                         eng.lower_ap(nc.const_aps.scalar_like(0.0, in_)),
                         mybir.ImmediateValue(dtype=mybir.dt.float32,
                                              value=1.0),
                         mybir.ImmediateValue(dtype=mybir.dt.float32,
                                              value=0.0)],
                    outs=[eng.lower_ap(out)],
                ))

            # G @ Wk / G @ Wq for both batches in ONE matmul each:
            # lhsT = gsum_bf [64K, 128M] (G0 | G1 side by side).
            gwk_ps = pspool.tile([128, 512], f32, tag="tps", bufs=3,
                                 name="gwk_ps")
            nc.tensor.matmul(gwk_ps[:], gsum_bf[:], wk2_s[0:64, :],
                             start=True, stop=True)
            gwq_ps = pspool.tile([128, 512], f32, tag="tps", bufs=3,
                                 name="gwq_ps")
            nc.tensor.matmul(gwq_ps[:], gsum_bf[:], wq2_s[0:64, :],
                             start=True, stop=True)
            pk = spool.tile([128, 512], bf16, tag="pk", name="pk")
            nc.vector.tensor_mul(pk[:], wk2_s[:], gwk_ps[:])
            pq = spool.tile([128, 512], bf16, tag="pq", name="pq")
            nc.vector.tensor_mul(pq[:], wq2_s[:], gwq_ps[:])
            gwq_sb = spool.tile([128, 512], bf16, tag="gwq_sb",
                                name="gwq_sb")
            nc.scalar.copy(gwq_sb[:], gwq_ps[:])

            # column sums kk/qq via all-ones lhsT (per batch quadrant)
            kk_ps = pspool.tile([128, 512], f32, tag="tps", bufs=3,
                                name="kk_ps")
            for b in range(B):
                nc.tensor.matmul(kk_ps[b * 64:(b + 1) * 64, :],
                                 ones2_s[b * 64:(b + 1) * 64, :],
                                 pk[b * 64:(b + 1) * 64, :],
                                 start=True, stop=True,
                                 skip_group_check=True,
                                 tile_position=(b * 64, b * 64))
            invk = spool.tile([128, 512], bf16, tag="invk", name="invk")
            act_rsqrt(invk[:], kk_ps[:])
            qq_ps = pspool.tile([128, 512], f32, tag="tps", bufs=3,
                                name="qq_ps")
            for b in range(B):
                nc.tensor.matmul(qq_ps[b * 64:(b + 1) * 64, :],
                                 ones2_s[b * 64:(b + 1) * 64, :],
                                 pq[b * 64:(b + 1) * 64, :],
                                 start=True, stop=True,
                                 skip_group_check=True,
                                 tile_position=(b * 64, b * 64))
            iqs = spool.tile([128, 512], f32, tag="iqs", name="iqs")
            act_rsqrt(iqs[:], qq_ps[:])
            invq = spool.tile([128, 512], bf16, tag="invq", name="invq")
            nc.vector.tensor_mul(invq[:], iqs[:], rsc2_s[:])

            # scale matrix: K=1 outer products invk_h (x) invq_h per batch
            scl_ps = pspool.tile([128, 512], f32, tag="tps", bufs=3,
                                 name="scl_ps")
            for b in range(B):
                for h in range(8):
                    nc.tensor.matmul(
                        scl_ps[b * 64:(b + 1) * 64, h * 64:(h + 1) * 64],
                        invk[b * 64:b * 64 + 1, h * 64:(h + 1) * 64],
                        invq[b * 64:b * 64 + 1, h * 64:(h + 1) * 64],
                        start=True, stop=True, skip_group_check=True,
                        tile_position=(b * 64, b * 64))
            scl_sb = spool.tile([128, 512], f32, tag="scl_sb",
                                name="scl_sb")
            nc.scalar.copy(scl_sb[:], scl_ps[:])

            # raw scores Wk_h^T (G Wq)_h per batch quadrant
            s_ps = pspool.tile([128, 512], f32, tag="tps", bufs=3,
                               name="s_ps")
            for b in range(B):
                for h in range(8):
                    nc.tensor.matmul(
                        s_ps[b * 64:(b + 1) * 64, h * 64:(h + 1) * 64],
                        wk2_s[b * 64:(b + 1) * 64, h * 64:(h + 1) * 64],
                        gwq_sb[b * 64:(b + 1) * 64, h * 64:(h + 1) * 64],
                        start=True, stop=True, skip_group_check=True,
                        tile_position=(b * 64, b * 64))
            expin = spool.tile([128, 512], f32, tag="expin", name="expin")
            nc.vector.tensor_mul(expin[:], s_ps[:], scl_sb[:])
            attn = spool.tile([128, 512], bf16, tag="attn", name="attn")
            nc.scalar.activation(attn[:], expin[:], Act.Exp)
            rs = spool.tile([128, 8], f32, tag="rs", name="rs")
            nc.vector.reduce_sum(
                rs[:], attn[:, :].rearrange("p (h e) -> p h e", h=8),
                axis=mybir.AxisListType.X)
            rsi = spool.tile([128, 8], f32, tag="rsi", name="rsi")
            nc.vector.reciprocal(rsi[:], rs[:])

            wps_t = spool.tile([128, 512], bf16, tag="wps_t", name="wps_t")
            for h in range(8):
                nc.scalar.mul(wps_t[:, h * 64:(h + 1) * 64],
                              wp2_s[:, h * 64:(h + 1) * 64],
                              rsi[:, h:h + 1])
            wt_ps = pspool.tile([128, 512], f32, tag="tps", bufs=3,
                                name="wt_ps")
            for b in range(B):
                for h in range(8):
                    nc.tensor.matmul(
                        wt_ps[b * 64:(b + 1) * 64, h * 64:(h + 1) * 64],
                        attn[b * 64:(b + 1) * 64, h * 64:(h + 1) * 64],
                        wps_t[b * 64:(b + 1) * 64, h * 64:(h + 1) * 64],
                        start=True, stop=True, skip_group_check=True,
                        tile_position=(b * 64, b * 64))
            # wt_sb2: per batch a [128,256] K-major grid (heads 2-up)
            wt_sb2 = spool.tile([128, 512], bf16, tag="wt_sb2",
                                name="wt_sb2")
            for b in range(B):
                for h in range(8):
                    nc.scalar.copy(
                        wt_sb2[(h % 2) * 64:(h % 2) * 64 + 64,
                               b * 256 + (h // 2) * 64:
                               b * 256 + (h // 2) * 64 + 64],
                        wt_ps[b * 64:(b + 1) * 64, h * 64:(h + 1) * 64])
            weff_ps = pspool.tile([128, 64], f32, tag="tps", bufs=3,
                                  name="weff_ps")
            for b in range(B):
                for k in range(4):
                    nc.tensor.matmul(
                        weff_ps[b * 64:(b + 1) * 64, :],
                        wvt_s[:, k * 64:(k + 1) * 64],
                        wt_sb2[:, b * 256 + k * 64:b * 256 + (k + 1) * 64],
                        start=(k == 0), stop=(k == 3),
                        skip_group_check=True,
                        tile_position=(0, b * 64))
            # block-diagonal [Weff_b0 ; Weff_b1] stationary for the attention
            # term (partition-aligned copies: b1 already lives on 64:127)
            c2 = spool.tile([128, 128], bf16, tag="ctr2", name="ctr2")
            nc.vector.memset(c2[:], 0.0)
            nc.scalar.copy(c2[0:64, 0:64], weff_ps[0:64, :])
            nc.scalar.copy(c2[64:128, 64:128], weff_ps[64:128, :])
            ctr2.append(c2)

            # ---- tail: folded late pairs (single osb write + flush), then
            # early pairs' attention fix-ups + flushes
            for ci in range(N_EARLY, NCHUNKS):
                gen1(ci)
            for ci in sorted(early):
                gen2(ci)

    nc.compile()
    return nc


def _prep_static(Wq, Wk, Wv, rescale, Wp, bp, pos_k):
    pk = np.asarray(pos_k, np.float32).reshape(C, 3, 3)
    eye = np.eye(C, dtype=np.float32)
    tap9 = np.zeros((128, 9 * 128), np.float32)
    for dy in range(3):
        for dx in range(3):
            blk = np.zeros((128, 128), np.float32)
            blk[0:64, 0:64] = eye * pk[:, dy, dx]
            blk[64:128, 64:128] = eye * pk[:, dy, dx]
            s = (dy * 3 + dx) * 128
            tap9[:, s:s + 128] = blk
    wvt = np.ascontiguousarray(
        np.asarray(Wv, np.float32).T.reshape(4, 128, 64)
        .transpose(1, 0, 2).reshape(128, 256))
    wp = np.ascontiguousarray(
        np.asarray(Wp, np.float32).reshape(8, 64, 64)
        .transpose(1, 0, 2).reshape(64, 512))
    rsc = np.broadcast_to(
        np.repeat(np.asarray(rescale, np.float32).ravel(), 64),
        (C, INNER)).astype(np.float32)
    wqf = np.asarray(Wq, np.float32)
    wkf = np.asarray(Wk, np.float32)
    return {
        "wq2": np.vstack([wqf, wqf]).astype(BF),
        "wk2": np.vstack([wkf, wkf]).astype(BF),
        "wvt": wvt.astype(BF),
        "wp2": np.ascontiguousarray(np.vstack([wp, wp])).astype(np.float32),
        "tap9": tap9.astype(BF),
        "ones2": np.ones((128, C), BF),
        "idn": np.eye(128, dtype=np.float32).astype(BF),
        "bp": np.tile(np.asarray(bp, np.float32), B).reshape(128, 1),
        "rsc2": np.ascontiguousarray(np.vstack([rsc, rsc])).astype(
            np.float32),
    }


def _install_ntff_hook():
    """Recreate the antenv.axon_hooks NTFF profiling hook the boot skipped
    (the container's antenv stub lacks axon_hooks).  Profiling only."""
    import sys
    import ctypes
    import contextlib
    import types

    if "antenv.axon_hooks" in sys.modules:
        return
    so_path = "/opt/axon/libaxon_pjrt.so"
    lib = ctypes.CDLL(so_path)
    if not hasattr(lib, "axon_start_nrt_profile"):
        return
    lib.axon_start_nrt_profile.argtypes = [ctypes.POINTER(ctypes.c_int64),
                                           ctypes.c_size_t]
    lib.axon_start_nrt_profile.restype = ctypes.c_int64
    lib.axon_stop_nrt_profile.argtypes = [ctypes.c_char_p]
    lib.axon_stop_nrt_profile.restype = ctypes.c_int64

    @contextlib.contextmanager
    def _hook(output_dir, device_ids):
        import jax
        jax.devices()
        if device_ids:
            ids = (ctypes.c_int64 * len(device_ids))(*device_ids)
            rc = lib.axon_start_nrt_profile(ids, len(device_ids))
        else:
            rc = lib.axon_start_nrt_profile(None, 0)
        if rc != 0:
            raise RuntimeError(f"axon_start_nrt_profile rc={rc}")
        try:
            yield
        finally:
            n = lib.axon_stop_nrt_profile(str(output_dir).encode())
            print(f"profile: {n} ntff file(s) -> {output_dir}")

    mod = types.ModuleType("antenv.axon_hooks")
    mod.get_axon_ntff_profile_hook = lambda: _hook
    mod.set_axon_ntff_profile_hook = lambda h: None
    sys.modules["antenv.axon_hooks"] = mod

    import concourse.bass_utils as bu
    bu.upload_artifacts = lambda tmpdir: tmpdir


def kernel(x_in, Wq, Wk, Wv, rescale, Wp, bp, pos_k):
    from concourse.bass_utils import run_bass_kernel_spmd

    if "nc" not in _CACHE:
        _CACHE["nc"] = _build()
    nc = _CACHE["nc"]

    x_in = np.asarray(x_in, np.float32)
    static = _prep_static(Wq, Wk, Wv, rescale, Wp, bp, pos_k)

    # padded rows (1 halo each side), batch-stacked, bf16, 4 overlapping
    # 10-row slices per core
    HP = RPC + 2
    xp = np.zeros((B, C, H + 2, WP), np.float32)
    xp[:, :, 1:H + 1, 1:W + 1] = x_in
    in_maps = []
    for i in range(NCORES):
        slab = xp[:, :, i * RPC:i * RPC + HP, :]          # [2, 64, 34, 272]
        stk = slab.reshape(B * C, HP, WP)                 # [128, 34, 272]
        sl = np.concatenate(
            [stk[:, s:s + SL_ROWS, :].reshape(128, SL_FREE)
             for s in SL_START], axis=1)
        in_maps.append({"x": np.ascontiguousarray(sl).astype(BF), **static})

    trace = os.environ.get("KERNEL_PROFILE", "0") == "1"
    if trace:
        try:
            _install_ntff_hook()
        except Exception as e:
            print(f"ntff hook install failed: {e}")
            trace = False
    tmpdir = os.environ.get("KERNEL_TRACE_DIR") or None
    res = run_bass_kernel_spmd(nc, in_maps, core_ids=list(range(NCORES)),
                               trace=trace, tmpdir=tmpdir)
    _CACHE["exec_time_ns"] = res.exec_time_ns

    out = np.empty((B, C, H, W), np.float32)
    for i in range(NCORES):
        o = np.asarray(res.results[i]["out"]).astype(np.float32)
        out[:, :, i * RPC:(i + 1) * RPC, :] = o.reshape(B, C, RPC, W)
    return out


# revision 10
# speedup vs baseline: 1.0451x; 1.0451x over previous
"""Distributed Trainium2 kernel for the sparse-attention + depthwise-conv module.

Math: q/k are l2-normalized over the full spatial axis n and the score matrix
is a tiny [b,h,64,64], so the whole attention collapses through the per-batch
Gram matrix G = X^T X ([64,64]):
  S_raw[h] = Wk_h^T G Wq_h, kk = diag(Wk_h^T G Wk_h), qq = diag(Wq_h^T G Wq_h)
  attn = softmax(S_raw * rescale / sqrt(kk qq))
  Wtilde[h] = attn_h^T (Wp_h / rowsum),  Weff = Wv @ Wtilde   ([64,64] per b)
  out = depthwise_conv3x3(x) + X @ Weff + bp
Only G crosses cores (AllGather of 16KB bf16 + local sum: the 8-core AllGather
floor is ~2x lower than AllReduce's, and the payload is latency-bound).

Layout: x is uploaded batch-STACKED bf16 ([128part = b0 ch | b1 ch]) in four
10-row slices (2-row overlap) so transposes/conv matmuls start as slices land.
 - G: PE pair-transposes [128,128] blocks of a single image row (both batches
   at once) -> xt; G MMs accumulate b0/b1 concurrently in opposite PE column
   groups (g_ps [128,64]).
 - conv: block-diagonal [128,128] stationaries (diag(tap) per batch) compute
   both batches' 64 output channels in one K=128 matmul; 9 tap slots per
   2-row chunk + a folded Weff slot (late pairs) or a separate fix-up matmul
   + DVE add (early pairs).  Early pairs' conv work is AllGather-independent
   and fills the PE through the collective's ~20-35us latency, also keeping
   the HAM clock-gate warm (2.4GHz).
PE warmup matmuls on a zero tile run during the x DMA window so the clock
gate is already released when real work arrives.
"""

import os
import numpy as np
import ml_dtypes

BF = ml_dtypes.bfloat16
B, C, H, W = 2, 64, 256, 256
HEADS, D = 8, 64
INNER = HEADS * D          # 512
NCORES = 8
RPC = H // NCORES          # 32 output rows per core per batch
WP = 272                   # padded row length (16-elem multiple)
NLOC = RPC * W             # 8192 spatial positions per core per batch
NCHUNKS = NLOC // 512      # 16 row-pair chunks
SL_START = [0, 8, 16, 24]  # slice start rows (10 rows each, 2-row overlap)
SL_ROWS = 10
SL_FREE = SL_ROWS * WP     # 2720

_CACHE = {}


def _build():
    import concourse.bass as bass
    import concourse.bacc as bacc
    import concourse.mybir as mybir
    import concourse.tile as tile

    f32 = mybir.dt.float32
    bf16 = mybir.dt.bfloat16

    nc = bacc.Bacc("TRN2", target_bir_lowering=False, debug=False,
                   num_devices=NCORES)

    x_d = nc.dram_tensor("x", [128, 4 * SL_FREE], bf16,
                         kind="ExternalInput").ap()
    wq2_d = nc.dram_tensor("wq2", [128, INNER], bf16,
                           kind="ExternalInput").ap()
    wk2_d = nc.dram_tensor("wk2", [128, INNER], bf16,
                           kind="ExternalInput").ap()
    wvt_d = nc.dram_tensor("wvt", [128, 256], bf16, kind="ExternalInput").ap()
    wp2_d = nc.dram_tensor("wp2", [128, INNER], f32,
                           kind="ExternalInput").ap()
    tap9_d = nc.dram_tensor("tap9", [128, 9 * 128], bf16,
                            kind="ExternalInput").ap()
    ones2_d = nc.dram_tensor("ones2", [128, C], bf16,
                             kind="ExternalInput").ap()
    idn_d = nc.dram_tensor("idn", [128, 128], bf16, kind="ExternalInput").ap()
    bp_d = nc.dram_tensor("bp", [128, 1], f32, kind="ExternalInput").ap()
    rsc2_d = nc.dram_tensor("rsc2", [128, INNER], f32,
                            kind="ExternalInput").ap()
    out_d = nc.dram_tensor("out", [B * C, NLOC], bf16,
                           kind="ExternalOutput").ap()

    Act = mybir.ActivationFunctionType
    N_EARLY = int(os.environ.get("KERNEL_EARLY_PAIRS", "16"))
    N_WARM = int(os.environ.get("KERNEL_WARM_MMS", "55"))
    N_FILL = int(os.environ.get("KERNEL_FILL_MMS", "110"))

    with tile.TileContext(nc) as tc:
        with (
            tc.tile_pool(name="xp", bufs=1) as xpool,
            tc.tile_pool(name="wp", bufs=1) as wpool,
            tc.tile_pool(name="sp", bufs=1) as spool,
            tc.tile_pool(name="xt", bufs=4) as xtpool,
            tc.tile_pool(name="ob", bufs=1) as opool,
            tc.tile_pool(name="ps", bufs=1, space="PSUM") as pspool,
            tc.tile_pool(name="dr", bufs=1, space="DRAM") as drpool,
        ):
            # ---- PE warmup: ~40 small matmuls on a zeroed tile release the
            # HAM clock gate (~3.4us of activity) while the x DMA streams in.
            warm = spool.tile([64, 64], bf16, tag="warm")
            nc.vector.memset(warm[:], 0.0)
            warm_ps = pspool.tile([64, 64], f32, tag="tps", bufs=4,
                                  name="warm_ps")
            for i in range(N_WARM):
                nc.tensor.matmul(warm_ps[:], warm[:], warm[:],
                                 start=(i == 0), stop=(i == N_WARM - 1))

            # ---- x: 4 batch-stacked bf16 slices (10 rows each)
            slices = []
            for g in range(4):
                s = xpool.tile([128, SL_FREE], bf16, tag=f"s{g}",
                               name=f"s{g}")
                nc.gpsimd.dma_start(s[:], x_d[:, g * SL_FREE:(g + 1) * SL_FREE])
                slices.append(s)
            svs = [s[:, :].rearrange("p (r w) -> p r w", w=WP)
                   for s in slices]

            # ---- weights (idn gates the transposes -> first)
            idn_s = wpool.tile_from(idn_d)
            tap9_s = wpool.tile_from(tap9_d)
            bp_s = wpool.tile_from(bp_d)
            wq2_s = wpool.tile_from(wq2_d)
            wk2_s = wpool.tile_from(wk2_d)
            wvt_s = wpool.tile_from(wvt_d)
            wp2_s = wpool.tile_from(wp2_d)
            ones2_s = wpool.tile_from(ones2_d)
            rsc2_s = wpool.tile_from(rsc2_d)

            # ---- G = X^T X for both batches at once.
            # Transpose one image row x 128 cols of the stacked tile
            # ([128part=(b0|b1 ch), 128]) against I128; 4 transposes fill a
            # [128,512] psum -> bf16 xt; 8 G-matmuls per xt (4 blocks x 2
            # batches) accumulate b0 into PE cols 0:63 / b1 into 64:127
            # concurrently (opposite column groups, separate psum halves).
            def row_slice(r):
                g = max(0, (r - 2) // 8)
                return g, r - SL_START[g]

            g_ps = pspool.tile([128, 64], f32, tag="gps", name="g_ps")
            first = {0: True, 1: True}
            for grp in range(16):
                tp = pspool.tile([128, 512], f32, tag="tps", bufs=4,
                                 name=f"tp{grp}")
                for j in range(4):
                    t = grp * 4 + j          # 0..63
                    r = 1 + t // 2
                    xh = t % 2
                    g, lr = row_slice(r)
                    off = lr * WP + 1 + 128 * xh
                    nc.tensor.matmul(tp[:, j * 128:(j + 1) * 128],
                                     slices[g][:, off:off + 128], idn_s[:],
                                     start=True, stop=True,
                                     skip_group_check=True)
                xt = xtpool.tile([128, 512], bf16, tag="xt",
                                 name=f"xt{grp}")
                if grp % 2 == 0:
                    nc.vector.tensor_copy(xt[:], tp[:])
                else:
                    nc.scalar.copy(xt[:], tp[:])
                for j in range(4):
                    for b in range(B):
                        nc.tensor.matmul(
                            g_ps[b * 64:(b + 1) * 64, :],
                            xt[:, j * 128 + b * 64:j * 128 + (b + 1) * 64],
                            xt[:, j * 128 + b * 64:j * 128 + (b + 1) * 64],
                            start=first[b],
                            stop=(grp == 15 and j == 3),
                            skip_group_check=True,
                            tile_position=(0, b * 64),
                        )
                        first[b] = False

            # ---- AllGather G (bf16, 16KB) + local sum.  gcat[64,128] holds
            # b0|b1 side by side (ACT copies handle the partition crossing).
            gcat = spool.tile([64, 128], bf16, tag="gcat")
            nc.scalar.copy(gcat[:, 0:64], g_ps[0:64, :])
            nc.scalar.copy(gcat[:, 64:128], g_ps[64:128, :])
            g_in = drpool.tile([64, 128], bf16, tag="gin")
            g_out = drpool.tile([NCORES, 64, 128], bf16, tag="gout")
            nc.sync.dma_start(g_in[:], gcat[:])
            nc.gpsimd.collective_compute(
                "AllGather", mybir.AluOpType.bypass,
                replica_groups=[list(range(NCORES))],
                ins=[g_in.opt()], outs=[g_out.opt()],
            )
            g8 = spool.tile([64, NCORES * 128], bf16, tag="g8")
            nc.sync.dma_start(
                g8[:, :].rearrange("p (r c) -> p r c", c=128),
                g_out.rearrange("r p c -> p r c"))
            # local sum of the 8 gathered G blocks: 3-level add tree
            g4 = spool.tile([64, 512], bf16, tag="g4")
            nc.vector.tensor_add(g4[:], g8[:, 0:512], g8[:, 512:1024])
            g2 = spool.tile([64, 256], bf16, tag="g2")
            nc.vector.tensor_add(g2[:], g4[:, 0:256], g4[:, 256:512])
            gsum_bf = spool.tile([64, 128], bf16, tag="gsumbf")
            nc.vector.tensor_add(gsum_bf[:], g2[:, 0:128], g2[:, 128:256])

            # ---- conv (+ attention) chunk pairs: block-diagonal [128,128]
            # stationaries compute both batches' 64 out-channels in one
            # K=128 matmul.  9 tap slots; late pairs fold X@Weff as a 10th
            # slot, early pairs get a separate fix-up matmul + DVE add.
            ctr2 = []        # filled after head math
            osbs = {}
            early = set(range(N_EARLY))

            def gen1(ci):
                folded = ci not in early
                g = ci // 4
                ly0 = 2 * ci - SL_START[g]
                cps = pspool.tile([128, 512], f32, tag="conv", bufs=3,
                                  name=f"cps{ci}")
                nslot = 10 if folded else 9
                si = 0
                for dy in range(3):
                    for dx in range(3):
                        nc.tensor.matmul(
                            cps[:],
                            tap9_s[:, (dy * 3 + dx) * 128:
                                   (dy * 3 + dx + 1) * 128],
                            svs[g][:, ly0 + dy:ly0 + dy + 2, dx:dx + 256],
                            start=(si == 0), stop=(si == nslot - 1),
                            skip_group_check=True)
                        si += 1
                if folded:
                    nc.tensor.matmul(
                        cps[:], ctr2[0][:],
                        svs[g][:, ly0 + 1:ly0 + 3, 1:257],
                        start=False, stop=True, skip_group_check=True)
                q = ci // 4
                if q not in osbs:
                    osbs[q] = opool.tile([128, 2048], bf16, tag="osb",
                                         bufs=4, name=f"osb_q{q}")
                sl = osbs[q][:, (ci % 4) * 512:(ci % 4 + 1) * 512]
                nc.scalar.activation(sl, cps[:], Act.Identity,
                                     bias=bp_s[:])
                if folded and ci % 4 == 3:
                    flush(q)

            def gen2(ci):
                g = ci // 4
                ly0 = 2 * ci - SL_START[g]
                aps = pspool.tile([128, 512], f32, tag="conv", bufs=3,
                                  name=f"aps{ci}")
                nc.tensor.matmul(aps[:], ctr2[0][:],
                                 svs[g][:, ly0 + 1:ly0 + 3, 1:257],
                                 start=True, stop=True,
                                 skip_group_check=True)
                sl = osbs[ci // 4][:, (ci % 4) * 512:(ci % 4 + 1) * 512]
                nc.vector.tensor_add(sl, sl, aps[:])
                if ci % 4 == 3:
                    flush(ci // 4)

            def flush(q):
                nc.gpsimd.dma_start(out_d[:, q * 2048:(q + 1) * 2048],
                                    osbs[q][:])

            # AllGather-independent conv first: keeps the PE busy (and the
            # clock warm) through the collective's latency.
            for ci in sorted(early):
                gen1(ci)

            # ---- warm-fill: dependency-free matmuls sized to span the
            # AllGather's remaining latency so the HAM clock gate stays
            # released for the head chain + tail (env KERNEL_FILL_MMS).
            warm2 = spool.tile([64, 512], bf16, tag="warm2")
            nc.vector.memset(warm2[:], 0.0)
            fill_ps = pspool.tile([64, 512], f32, tag="tps", bufs=4,
                                  name="fill_ps")
            for i in range(N_FILL):
                nc.tensor.matmul(fill_ps[:], warm[:], warm2[:],
                                 start=(i == 0), stop=(i == N_FILL - 1))

            # ---- head math -> Weff, both batches merged: b0 lives on
            # partitions/PE-rows 0:63, b1 on 64:127 (doubled weights); each
            # elementwise/activation stage is ONE [128,512] op and the two
            # batches' matmuls run concurrently in opposite PE quadrants.
            def act_rsqrt(out, in_):
                # raw InstActivation: bass blocks ACT Rsqrt for accuracy, but
                # table accuracy (~1e-3) is far inside this kernel's 2e-2
                # budget and it replaces a 3.3us DVE Newton reciprocal.
                eng = nc.scalar
                return eng.add_instruction(mybir.InstActivation(
                    name=nc.get_next_instruction_name(),
                    func=Act.Rsqrt,
                    ins=[eng.lower_ap(in_),
                         eng.lower_ap(nc.const_aps.scalar_like(0.0, in_)),
                         mybir.ImmediateValue(dtype=mybir.dt.float32,
                                              value=1.0),
                         mybir.ImmediateValue(dtype=mybir.dt.float32,
                                              value=0.0)],
                    outs=[eng.lower_ap(out)],
                ))

            # G @ Wk / G @ Wq for both batches in ONE matmul each:
            # lhsT = gsum_bf [64K, 128M] (G0 | G1 side by side).
            gwk_ps = pspool.tile([128, 512], f32, tag="tps", bufs=4,
                                 name="gwk_ps")
            nc.tensor.matmul(gwk_ps[:], gsum_bf[:], wk2_s[0:64, :],
                             start=True, stop=True)
            gwq_ps = pspool.tile([128, 512], f32, tag="tps", bufs=4,
                                 name="gwq_ps")
            nc.tensor.matmul(gwq_ps[:], gsum_bf[:], wq2_s[0:64, :],
                             start=True, stop=True)
            pk = spool.tile([128, 512], bf16, tag="pk", name="pk")
            nc.vector.tensor_mul(pk[:], wk2_s[:], gwk_ps[:])
            pq = spool.tile([128, 512], bf16, tag="pq", name="pq")
            nc.vector.tensor_mul(pq[:], wq2_s[:], gwq_ps[:])

            # column sums kk/qq via all-ones lhsT (per batch quadrant)
            kk_ps = pspool.tile([128, 512], f32, tag="tps", bufs=4,
                                name="kk_ps")
            for b in range(B):
                nc.tensor.matmul(kk_ps[b * 64:(b + 1) * 64, :],
                                 ones2_s[b * 64:(b + 1) * 64, :],
                                 pk[b * 64:(b + 1) * 64, :],
                                 start=True, stop=True,
                                 skip_group_check=True,
                                 tile_position=(b * 64, b * 64))
            invk = spool.tile([128, 512], f32, tag="invk", name="invk")
            act_rsqrt(invk[:], kk_ps[:])
            qq_ps = pspool.tile([128, 512], f32, tag="tps", bufs=4,
                                name="qq_ps")
            for b in range(B):
                nc.tensor.matmul(qq_ps[b * 64:(b + 1) * 64, :],
                                 ones2_s[b * 64:(b + 1) * 64, :],
                                 pq[b * 64:(b + 1) * 64, :],
                                 start=True, stop=True,
                                 skip_group_check=True,
                                 tile_position=(b * 64, b * 64))
            iqs = spool.tile([128, 512], f32, tag="iqs", name="iqs")
            act_rsqrt(iqs[:], qq_ps[:])
            invq = spool.tile([128, 512], f32, tag="invq", name="invq")
            nc.vector.tensor_mul(invq[:], iqs[:], rsc2_s[:])
            # fold the normalizations into the score-matmul operands:
            # s_h = (wk_h invk_h)^T @ ((G wq)_h invq_h) so no separate
            # scale outer-product / multiply stages are needed
            wkn = spool.tile([128, 512], bf16, tag="wkn", name="wkn")
            nc.vector.tensor_mul(wkn[:], wk2_s[:], invk[:])
            gwqn = spool.tile([128, 512], bf16, tag="gwqn", name="gwqn")
            nc.vector.tensor_mul(gwqn[:], gwq_ps[:], invq[:])

            # normalized scores per batch quadrant, exp straight from PSUM
            s_ps = pspool.tile([128, 512], f32, tag="tps", bufs=4,
                               name="s_ps")
            for b in range(B):
                for h in range(8):
                    nc.tensor.matmul(
                        s_ps[b * 64:(b + 1) * 64, h * 64:(h + 1) * 64],
                        wkn[b * 64:(b + 1) * 64, h * 64:(h + 1) * 64],
                        gwqn[b * 64:(b + 1) * 64, h * 64:(h + 1) * 64],
                        start=True, stop=True, skip_group_check=True,
                        tile_position=(b * 64, b * 64))
            attn = spool.tile([128, 512], bf16, tag="attn", name="attn")
            nc.scalar.activation(attn[:], s_ps[:], Act.Exp)
            rs = spool.tile([128, 8], f32, tag="rs", name="rs")
            nc.vector.reduce_sum(
                rs[:], attn[:, :].rearrange("p (h e) -> p h e", h=8),
                axis=mybir.AxisListType.X)
            rsi = spool.tile([128, 8], f32, tag="rsi", name="rsi")
            nc.vector.reciprocal(rsi[:], rs[:])

            wps_t = spool.tile([128, 512], bf16, tag="wps_t", name="wps_t")
            for h in range(8):
                nc.vector.tensor_scalar_mul(wps_t[:, h * 64:(h + 1) * 64],
                                            wp2_s[:, h * 64:(h + 1) * 64],
                                            rsi[:, h:h + 1])
            wt_ps = pspool.tile([128, 512], f32, tag="tps", bufs=4,
                                name="wt_ps")
            for b in range(B):
                for h in range(8):
                    nc.tensor.matmul(
                        wt_ps[b * 64:(b + 1) * 64, h * 64:(h + 1) * 64],
                        attn[b * 64:(b + 1) * 64, h * 64:(h + 1) * 64],
                        wps_t[b * 64:(b + 1) * 64, h * 64:(h + 1) * 64],
                        start=True, stop=True, skip_group_check=True,
                        tile_position=(b * 64, b * 64))
            # wt_sb2: per batch a [128,256] K-major grid (heads 2-up)
            wt_sb2 = spool.tile([128, 512], bf16, tag="wt_sb2",
                                name="wt_sb2")
            for b in range(B):
                for h in range(8):
                    nc.vector.tensor_copy(
                        wt_sb2[(h % 2) * 64:(h % 2) * 64 + 64,
                               b * 256 + (h // 2) * 64:
                               b * 256 + (h // 2) * 64 + 64],
                        wt_ps[b * 64:(b + 1) * 64, h * 64:(h + 1) * 64])
            weff_ps = pspool.tile([128, 64], f32, tag="tps", bufs=4,
                                  name="weff_ps")
            for b in range(B):
                for k in range(4):
                    nc.tensor.matmul(
                        weff_ps[b * 64:(b + 1) * 64, :],
                        wvt_s[:, k * 64:(k + 1) * 64],
                        wt_sb2[:, b * 256 + k * 64:b * 256 + (k + 1) * 64],
                        start=(k == 0), stop=(k == 3),
                        skip_group_check=True,
                        tile_position=(0, b * 64))
            # block-diagonal [Weff_b0 ; Weff_b1] stationary for the attention
            # term (partition-aligned copies: b1 already lives on 64:127)
            c2 = spool.tile([128, 128], bf16, tag="ctr2", name="ctr2")
            nc.vector.memset(c2[:], 0.0)
            nc.scalar.copy(c2[0:64, 0:64], weff_ps[0:64, :])
            nc.scalar.copy(c2[64:128, 64:128], weff_ps[64:128, :])
            ctr2.append(c2)

            # ---- tail: folded late pairs (single osb write + flush), then
            # early pairs' attention fix-ups + flushes
            for ci in range(N_EARLY, NCHUNKS):
                gen1(ci)
            for ci in sorted(early):
                gen2(ci)

    nc.compile()
    return nc


def _prep_static(Wq, Wk, Wv, rescale, Wp, bp, pos_k):
    pk = np.asarray(pos_k, np.float32).reshape(C, 3, 3)
    eye = np.eye(C, dtype=np.float32)
    tap9 = np.zeros((128, 9 * 128), np.float32)
    for dy in range(3):
        for dx in range(3):
            blk = np.zeros((128, 128), np.float32)
            blk[0:64, 0:64] = eye * pk[:, dy, dx]
            blk[64:128, 64:128] = eye * pk[:, dy, dx]
            s = (dy * 3 + dx) * 128
            tap9[:, s:s + 128] = blk
    wvt = np.ascontiguousarray(
        np.asarray(Wv, np.float32).T.reshape(4, 128, 64)
        .transpose(1, 0, 2).reshape(128, 256))
    wp = np.ascontiguousarray(
        np.asarray(Wp, np.float32).reshape(8, 64, 64)
        .transpose(1, 0, 2).reshape(64, 512))
    rsc = np.broadcast_to(
        np.repeat(np.asarray(rescale, np.float32).ravel(), 64),
        (C, INNER)).astype(np.float32)
    wqf = np.asarray(Wq, np.float32)
    wkf = np.asarray(Wk, np.float32)
    return {
        "wq2": np.vstack([wqf, wqf]).astype(BF),
        "wk2": np.vstack([wkf, wkf]).astype(BF),
        "wvt": wvt.astype(BF),
        "wp2": np.ascontiguousarray(np.vstack([wp, wp])).astype(np.float32),
        "tap9": tap9.astype(BF),
        "ones2": np.ones((128, C), BF),
        "idn": np.eye(128, dtype=np.float32).astype(BF),
        "bp": np.tile(np.asarray(bp, np.float32), B).reshape(128, 1),
        "rsc2": np.ascontiguousarray(np.vstack([rsc, rsc])).astype(
            np.float32),
    }


def _install_ntff_hook():
    """Recreate the antenv.axon_hooks NTFF profiling hook the boot skipped
    (the container's antenv stub lacks axon_hooks).  Profiling only."""
    import sys
    import ctypes
    import contextlib
    import types

    if "antenv.axon_hooks" in sys.modules:
        return
    so_path = "/opt/axon/libaxon_pjrt.so"
    lib = ctypes.CDLL(so_path)
    if not hasattr(lib, "axon_start_nrt_profile"):
        return
    lib.axon_start_nrt_profile.argtypes = [ctypes.POINTER(ctypes.c_int64),
                                           ctypes.c_size_t]
    lib.axon_start_nrt_profile.restype = ctypes.c_int64
    lib.axon_stop_nrt_profile.argtypes = [ctypes.c_char_p]
    lib.axon_stop_nrt_profile.restype = ctypes.c_int64

    @contextlib.contextmanager
    def _hook(output_dir, device_ids):
        import jax
        jax.devices()
        if device_ids:
            ids = (ctypes.c_int64 * len(device_ids))(*device_ids)
            rc = lib.axon_start_nrt_profile(ids, len(device_ids))
        else:
            rc = lib.axon_start_nrt_profile(None, 0)
        if rc != 0:
            raise RuntimeError(f"axon_start_nrt_profile rc={rc}")
        try:
            yield
        finally:
            n = lib.axon_stop_nrt_profile(str(output_dir).encode())
            print(f"profile: {n} ntff file(s) -> {output_dir}")

    mod = types.ModuleType("antenv.axon_hooks")
    mod.get_axon_ntff_profile_hook = lambda: _hook
    mod.set_axon_ntff_profile_hook = lambda h: None
    sys.modules["antenv.axon_hooks"] = mod

    import concourse.bass_utils as bu
    bu.upload_artifacts = lambda tmpdir: tmpdir


def kernel(x_in, Wq, Wk, Wv, rescale, Wp, bp, pos_k):
    from concourse.bass_utils import run_bass_kernel_spmd

    if "nc" not in _CACHE:
        _CACHE["nc"] = _build()
    nc = _CACHE["nc"]

    x_in = np.asarray(x_in, np.float32)
    static = _prep_static(Wq, Wk, Wv, rescale, Wp, bp, pos_k)

    # padded rows (1 halo each side), batch-stacked, bf16, 4 overlapping
    # 10-row slices per core
    HP = RPC + 2
    xp = np.zeros((B, C, H + 2, WP), np.float32)
    xp[:, :, 1:H + 1, 1:W + 1] = x_in
    in_maps = []
    for i in range(NCORES):
        slab = xp[:, :, i * RPC:i * RPC + HP, :]          # [2, 64, 34, 272]
        stk = slab.reshape(B * C, HP, WP)                 # [128, 34, 272]
        sl = np.concatenate(
            [stk[:, s:s + SL_ROWS, :].reshape(128, SL_FREE)
             for s in SL_START], axis=1)
        in_maps.append({"x": np.ascontiguousarray(sl).astype(BF), **static})

    trace = os.environ.get("KERNEL_PROFILE", "0") == "1"
    if trace:
        try:
            _install_ntff_hook()
        except Exception as e:
            print(f"ntff hook install failed: {e}")
            trace = False
    tmpdir = os.environ.get("KERNEL_TRACE_DIR") or None
    res = run_bass_kernel_spmd(nc, in_maps, core_ids=list(range(NCORES)),
                               trace=trace, tmpdir=tmpdir)
    _CACHE["exec_time_ns"] = res.exec_time_ns

    out = np.empty((B, C, H, W), np.float32)
    for i in range(NCORES):
        o = np.asarray(res.results[i]["out"]).astype(np.float32)
        out[:, :, i * RPC:(i + 1) * RPC, :] = o.reshape(B, C, RPC, W)
    return out
